# revision 2
# baseline (speedup 1.0000x reference)
"""OS-CFAR 2D rank filter (rank-36-of-144 ring) on 8 Trainium2 NeuronCores.

Strategy: spatially shard [512,1024] into 8 tiles of [128,512] (4 row-bands
x 2 col-halves) with 6-wide circular halos.

Per core, instead of per-pixel max8/match_replace rounds (vector-engine
bound at ~1890ns/pixel-col), compute the order statistic with a shared
sorted-column merge network operating on "planes" (dy-shifted fp16 images),
where every elementwise min/max instruction processes all 128x524 pixels at
once and fp16 gets the DVE 2x perf mode:

  D[dy]    = fp16(ALPHA * slab[p+dy, x])        13 leaf planes
  T4/B4/M5 = vertical sorts of ring column segments (top4/bot4/mid5)
  S8h      = merge(T4,B4)      sorted holed column (8)
  H2/H4    = dyadic merges of holed columns at x..x+3
  H5       = top36 of holed cols x..x+4 = merge(H4, S8h@4)
  MM2/MMM4 = dyadic merges of mid segments
  M4       = top36 of full cols x..x+3 = merge(H4, MMM4)
  FLR      = top36 of the 8 full cols = merge(M4, M4@9)
  ans      = rank36(FLR u H5@4) via max_{i+j=36} min(FLR_i, H5_j)

The rank-36 value of the fp16-rounded multiset equals fp16(rank-36 of the
originals) because rounding is monotone; rel err <= 2^-11 vs the f32
reference, far inside the 2e-2 gate.
"""

import math
from collections import defaultdict
from dataclasses import dataclass

import numpy as np

# ---------------------------------------------------------------- constants
G = (2, 2)
T = (4, 4)
PFA = 1e-05
K = 108
N = 144
PW = 6
V, R = 512, 1024
SLAB_H, SLAB_W = 140, 524      # 128 + 2*PW, 512 + 2*PW
RANK = 36                      # need the 36th largest of the 144 ring cells
SLOTW = 524


def _log_factorial(n):
    n = n + 1
    if n < 9:
        return np.log(float(math.factorial(n)))
    return 0.5 * (np.log(2 * np.pi) - np.log(n)) + n * (
        np.log(n + 1.0 / (12.0 * n - 1.0 / (10.0 * n))) - 1.0
    )


def _fun(k, n, t, pfa):
    return (
        _log_factorial(n)
        - _log_factorial(n - k)
        - np.sum(np.log(np.arange(n, n - k, -1) + t))
        - np.log(pfa)
    )


def _os_cfar_threshold(k, n, pfa):
    lo, hi = 1.0, 1e32
    for _ in range(300):
        mid = 0.5 * (lo + hi)
        if _fun(k, n, mid, pfa) > 0:
            lo = mid
        else:
            hi = mid
    return 0.5 * (lo + hi)


ALPHA = float(np.float32(_os_cfar_threshold(K, N, PFA)))

_CACHE = {}

# ------------------------------------------------------- network generation


@dataclass
class _Op:
    kind: str                 # 'min' | 'max' | 'leaf'
    out: int
    a: int = -1
    ao: int = 0
    b: int = -1
    bo: int = 0
    stage: str = ""


class _Net:
    def __init__(self):
        self.ops = []
        self.n_planes = 0
        self.leaf_meta = {}

    def leaf(self, dy):
        p = self.n_planes
        self.n_planes += 1
        self.leaf_meta[p] = dy
        self.ops.append(_Op("leaf", p, stage="leaf"))
        return p

    def emit(self, kind, a, ao, b, bo, stage):
        p = self.n_planes
        self.n_planes += 1
        self.ops.append(_Op(kind, p, a, ao, b, bo, stage))
        return p

    def ce(self, a, b, stage):
        hi = self.emit("max", a, 0, b, 0, stage)
        lo = self.emit("min", a, 0, b, 0, stage)
        return hi, lo


def _sort_net(net, planes, stage):
    n = len(planes)
    pairs = {
        4: [(0, 1), (2, 3), (0, 2), (1, 3), (1, 2)],
        5: [(0, 1), (3, 4), (2, 4), (2, 3), (0, 3), (0, 2), (1, 4), (1, 3), (1, 2)],
    }[n]
    w = list(planes)
    for i, j in pairs:
        hi, lo = net.ce(w[i], w[j], stage)
        w[i], w[j] = hi, lo
    return w


def _oem(net, A, B, stage):
    if not A:
        return list(B)
    if not B:
        return list(A)
    if len(A) == 1 and len(B) == 1:
        (pa, oa), (pb, ob) = A[0], B[0]
        hi = net.emit("max", pa, oa, pb, ob, stage)
        lo = net.emit("min", pa, oa, pb, ob, stage)
        return [(hi, 0), (lo, 0)]
    E = _oem(net, A[0::2], B[0::2], stage)
    O = _oem(net, A[1::2], B[1::2], stage)
    res = [E[0]]
    n_pairs = min(len(O), len(E) - 1)
    for i in range(n_pairs):
        (po, oo), (pe, oe) = O[i], E[i + 1]
        hi = net.emit("max", po, oo, pe, oe, stage)
        lo = net.emit("min", po, oo, pe, oe, stage)
        res += [(hi, 0), (lo, 0)]
    if len(O) > n_pairs:
        res += O[n_pairs:]
    else:
        res += E[n_pairs + 1:]
    return res


def _at(planes, off):
    return [(p, o + off) for (p, o) in planes]


def _net_build():
    net = _Net()
    D = [net.leaf(dy) for dy in range(13)]

    T4 = _sort_net(net, D[0:4], "vsort")
    B4 = _sort_net(net, D[9:13], "vsort")
    M5 = _sort_net(net, D[4:9], "vsort")
    S8h = _oem(net, [(p, 0) for p in T4], [(p, 0) for p in B4], "vs8")
    M5l = [(p, 0) for p in M5]

    H2 = _oem(net, S8h, _at(S8h, 1), "H2")
    H4 = _oem(net, H2, _at(H2, 2), "H4")
    H5 = _oem(net, H4, _at(S8h, 4), "H5")[:RANK]

    MM2 = _oem(net, M5l, _at(M5l, 1), "MM2")
    MMM4 = _oem(net, MM2, _at(MM2, 2), "MMM4")
    M4 = _oem(net, H4, MMM4, "M4")[:RANK]
    FLR = _oem(net, M4, _at(M4, 9), "FLR")[:RANK]

    terms = []
    for i in range(RANK + 1):
        j = RANK - i
        if i == 0:
            terms.append((H5[j - 1][0], H5[j - 1][1] + 4))
        elif j == 0:
            terms.append(FLR[i - 1])
        else:
            (pa, oa) = FLR[i - 1]
            (pb, ob) = H5[j - 1]
            t = net.emit("min", pa, oa, pb, ob + 4, "fin_min")
            terms.append((t, 0))
    while len(terms) > 1:
        nxt = []
        for k in range(0, len(terms) - 1, 2):
            (pa, oa), (pb, ob) = terms[k], terms[k + 1]
            nxt.append((net.emit("max", pa, oa, pb, ob, "fin_max"), 0))
        if len(terms) % 2:
            nxt.append(terms[-1])
        terms = nxt
    ans_plane, ans_off = terms[0]
    assert ans_off == 0
    return net, ans_plane


def _dce(net, ans):
    needed = {ans}
    for op in reversed(net.ops):
        if op.out in needed and op.kind != "leaf":
            needed.add(op.a)
            needed.add(op.b)
    return [op for op in net.ops if op.out in needed]


def _ranges(ops, ans, w_out):
    rng = {ans: (0, w_out)}
    for op in reversed(ops):
        if op.kind == "leaf":
            continue
        lo, hi = rng[op.out]
        for (src, off) in ((op.a, op.ao), (op.b, op.bo)):
            l, h = lo + off, hi + off
            if src in rng:
                l0, h0 = rng[src]
                rng[src] = (min(l0, l), max(h0, h))
            else:
                rng[src] = (l, h)
    return rng


# ------------------------------------------------------------- bass emission


def _build():
    import concourse.bass as bass
    import concourse.mybir as mybir
    from concourse.ap import AP

    f32 = mybir.dt.float32
    f16 = mybir.dt.float16

    net, ans = _net_build()
    ops = _dce(net, ans)
    rng = _ranges(ops, ans, 512)

    leaf_ops = [op for op in ops if op.kind == "leaf"]
    compute = [op for op in ops if op.kind != "leaf"]

    # leaves stored full width
    for op in leaf_ops:
        rng[op.out] = (0, SLOTW)

    # ---- slot allocation (greedy, free at last use) ----
    last_use = {}
    for idx, op in enumerate(compute):
        last_use[op.a] = idx
        last_use[op.b] = idx
    last_use[ans] = len(compute)          # read by the f32 convert at the end

    releases = defaultdict(list)
    for p, idx in last_use.items():
        releases[idx].append(p)

    slot_of = {}
    free = []
    next_slot = 0
    for op in leaf_ops:                   # leaves in dy order -> slots 0..12
        slot_of[op.out] = next_slot
        next_slot += 1
    for idx, op in enumerate(compute):
        if free:
            s = free.pop()
        else:
            s = next_slot
            next_slot += 1
        slot_of[op.out] = s
        for p in releases.get(idx, []):
            free.append(slot_of[p])
    nslot = next_slot
    arena_w = nslot * SLOTW

    nc = bass.Bass(trn_type="TRN2")
    slab = nc.dram_tensor("slab", [SLAB_H, SLAB_W], f32, kind="ExternalInput")
    out = nc.dram_tensor("out", [128, 512], f32, kind="ExternalOutput")

    REP_W = 13 * SLAB_W

    with (
        nc.sbuf_tensor([128, arena_w], f16) as arena,
        nc.sbuf_tensor([128, REP_W], f32) as rep,
        nc.sbuf_tensor([128, 512], f32) as ansf32,
        nc.semaphore() as dma_sem,
        nc.semaphore() as act_sem,
        nc.semaphore() as dve_sem,
        nc.Block() as block,
    ):

        def rd(p, off, lo_c, hi_c):
            """AP reading plane p at x+off for consumer range [lo_c, hi_c)."""
            a_lo, _ = rng[p]
            return AP(
                tensor=arena,
                offset=slot_of[p] * SLOTW + lo_c + off - a_lo,
                ap=[[arena_w, 128], [1, hi_c - lo_c]],
            )

        def wr(p):
            lo, hi = rng[p]
            return AP(
                tensor=arena,
                offset=slot_of[p] * SLOTW,
                ap=[[arena_w, 128], [1, hi - lo]],
            )

        @block.sync
        def _(sync):
            for g0 in range(0, 13, 4):
                gc = min(4, 13 - g0)
                src = AP(
                    tensor=slab,
                    offset=g0 * SLAB_W,
                    ap=[[SLAB_W, 128], [SLAB_W, gc], [1, SLAB_W]],
                )
                dst = AP(
                    tensor=rep,
                    offset=g0 * SLAB_W,
                    ap=[[REP_W, 128], [SLAB_W, gc], [1, SLAB_W]],
                )
                sync.dma_start(dst, src).then_inc(dma_sem, 16)
            sync.wait_ge(dve_sem, 1)
            sync.dma_start(out[:, :], ansf32[:, :]).then_inc(dma_sem, 16)

        @block.scalar
        def _(scalar):
            scalar.wait_ge(dma_sem, 16 * 4)
            for op in leaf_ops:
                dy = net.leaf_meta[op.out]
                ins = nc.scalar.activation(
                    out=wr(op.out),
                    in_=AP(
                        tensor=rep,
                        offset=dy * SLAB_W,
                        ap=[[REP_W, 128], [1, SLAB_W]],
                    ),
                    func=mybir.ActivationFunctionType.Copy,
                    scale=ALPHA,
                )
                ins.then_inc(act_sem, 1)

        @block.vector
        def _(vector):
            vector.wait_ge(act_sem, len(leaf_ops))
            alu = {"max": mybir.AluOpType.max, "min": mybir.AluOpType.min}
            for op in compute:
                lo, hi = rng[op.out]
                nc.vector.tensor_tensor(
                    out=wr(op.out),
                    in0=rd(op.a, op.ao, lo, hi),
                    in1=rd(op.b, op.bo, lo, hi),
                    op=alu[op.kind],
                )
            ins = nc.vector.tensor_copy(ansf32[:, :], rd(ans, 0, 0, 512))
            ins.then_inc(dve_sem, 1)

    return nc


def kernel(data: np.ndarray) -> np.ndarray:
    from concourse.bass_utils import run_bass_kernel_spmd

    img = np.asarray(data, dtype=np.float32)[0]          # [512,1024]
    pad = np.pad(img, PW, mode="wrap")                    # [524,1036]

    if "nc" not in _CACHE:
        _CACHE["nc"] = _build()
    nc = _CACHE["nc"]

    in_maps = []
    for c in range(8):
        band, half = c // 2, c % 2
        rb, cb = band * 128, half * 512
        in_maps.append(
            {"slab": np.ascontiguousarray(pad[rb: rb + SLAB_H, cb: cb + SLAB_W])}
        )

    res = run_bass_kernel_spmd(nc, in_maps, core_ids=list(range(8)))

    full = np.empty((V, R), dtype=np.float32)
    for c in range(8):
        band, half = c // 2, c % 2
        full[band * 128: (band + 1) * 128, half * 512: (half + 1) * 512] = (
            res.results[c]["out"]
        )
    return full


# revision 6
# speedup vs baseline: 1.0342x; 1.0342x over previous
"""OS-CFAR 2D rank filter (rank-36-of-144 ring) on 8 Trainium2 NeuronCores.

Strategy: spatially shard [512,1024] into 8 tiles of [128,512] (4 row-bands
x 2 col-halves) with 6-wide circular halos.

Per core, instead of per-pixel max8/match_replace rounds (vector-engine
bound at ~1890ns/pixel-col), compute the order statistic with a shared
sorted-column merge network operating on "planes" (dy-shifted fp16 images),
where every elementwise min/max instruction processes all 128x524 pixels at
once and fp16 gets the DVE 2x perf mode:

  D[dy]    = fp16(ALPHA * slab[p+dy, x])        13 leaf planes
  T4/B4/M5 = vertical sorts of ring column segments (top4/bot4/mid5)
  S8h      = merge(T4,B4)      sorted holed column (8)
  H2/H4    = dyadic merges of holed columns at x..x+3
  H5       = top36 of holed cols x..x+4 = merge(H4, S8h@4)
  MM2/MMM4 = dyadic merges of mid segments
  M4       = top36 of full cols x..x+3 = merge(H4, MMM4)
  FLR      = top36 of the 8 full cols = merge(M4, M4@9)
  ans      = rank36(FLR u H5@4) via max_{i+j=36} min(FLR_i, H5_j)

The rank-36 value of the fp16-rounded multiset equals fp16(rank-36 of the
originals) because rounding is monotone; rel err <= 2^-11 vs the f32
reference, far inside the 2e-2 gate.
"""

import math
from collections import defaultdict
from dataclasses import dataclass

import numpy as np

# ---------------------------------------------------------------- constants
G = (2, 2)
T = (4, 4)
PFA = 1e-05
K = 108
N = 144
PW = 6
V, R = 512, 1024
SLAB_H, SLAB_W = 140, 524      # 128 + 2*PW, 512 + 2*PW
RANK = 36                      # need the 36th largest of the 144 ring cells
SLOTW = 524


def _log_factorial(n):
    n = n + 1
    if n < 9:
        return np.log(float(math.factorial(n)))
    return 0.5 * (np.log(2 * np.pi) - np.log(n)) + n * (
        np.log(n + 1.0 / (12.0 * n - 1.0 / (10.0 * n))) - 1.0
    )


def _fun(k, n, t, pfa):
    return (
        _log_factorial(n)
        - _log_factorial(n - k)
        - np.sum(np.log(np.arange(n, n - k, -1) + t))
        - np.log(pfa)
    )


def _os_cfar_threshold(k, n, pfa):
    lo, hi = 1.0, 1e32
    for _ in range(300):
        mid = 0.5 * (lo + hi)
        if _fun(k, n, mid, pfa) > 0:
            lo = mid
        else:
            hi = mid
    return 0.5 * (lo + hi)


ALPHA = float(np.float32(_os_cfar_threshold(K, N, PFA)))

_CACHE = {}

# ------------------------------------------------------- network generation


@dataclass
class _Op:
    kind: str                 # 'min' | 'max' | 'leaf'
    out: int
    a: int = -1
    ao: int = 0
    b: int = -1
    bo: int = 0
    stage: str = ""


class _Net:
    def __init__(self):
        self.ops = []
        self.n_planes = 0
        self.leaf_meta = {}

    def leaf(self, dy):
        p = self.n_planes
        self.n_planes += 1
        self.leaf_meta[p] = dy
        self.ops.append(_Op("leaf", p, stage="leaf"))
        return p

    def emit(self, kind, a, ao, b, bo, stage):
        p = self.n_planes
        self.n_planes += 1
        self.ops.append(_Op(kind, p, a, ao, b, bo, stage))
        return p

    def ce(self, a, b, stage):
        hi = self.emit("max", a, 0, b, 0, stage)
        lo = self.emit("min", a, 0, b, 0, stage)
        return hi, lo


def _sort_net(net, planes, stage):
    n = len(planes)
    pairs = {
        4: [(0, 1), (2, 3), (0, 2), (1, 3), (1, 2)],
        5: [(0, 1), (3, 4), (2, 4), (2, 3), (0, 3), (0, 2), (1, 4), (1, 3), (1, 2)],
    }[n]
    w = list(planes)
    for i, j in pairs:
        hi, lo = net.ce(w[i], w[j], stage)
        w[i], w[j] = hi, lo
    return w


def _oem(net, A, B, stage):
    if not A:
        return list(B)
    if not B:
        return list(A)
    if len(A) == 1 and len(B) == 1:
        (pa, oa), (pb, ob) = A[0], B[0]
        hi = net.emit("max", pa, oa, pb, ob, stage)
        lo = net.emit("min", pa, oa, pb, ob, stage)
        return [(hi, 0), (lo, 0)]
    E = _oem(net, A[0::2], B[0::2], stage)
    O = _oem(net, A[1::2], B[1::2], stage)
    res = [E[0]]
    n_pairs = min(len(O), len(E) - 1)
    for i in range(n_pairs):
        (po, oo), (pe, oe) = O[i], E[i + 1]
        hi = net.emit("max", po, oo, pe, oe, stage)
        lo = net.emit("min", po, oo, pe, oe, stage)
        res += [(hi, 0), (lo, 0)]
    if len(O) > n_pairs:
        res += O[n_pairs:]
    else:
        res += E[n_pairs + 1:]
    return res


def _at(planes, off):
    return [(p, o + off) for (p, o) in planes]


def _net_build():
    net = _Net()
    D = [net.leaf(dy) for dy in range(13)]

    T4 = _sort_net(net, D[0:4], "vsort")
    B4 = _sort_net(net, D[9:13], "vsort")
    M5 = _sort_net(net, D[4:9], "vsort")
    S8h = _oem(net, [(p, 0) for p in T4], [(p, 0) for p in B4], "vs8")
    M5l = [(p, 0) for p in M5]

    H2 = _oem(net, S8h, _at(S8h, 1), "H2")
    H4 = _oem(net, H2, _at(H2, 2), "H4")
    H5 = _oem(net, H4, _at(S8h, 4), "H5")[:RANK]

    MM2 = _oem(net, M5l, _at(M5l, 1), "MM2")
    MMM4 = _oem(net, MM2, _at(MM2, 2), "MMM4")
    M4 = _oem(net, H4, MMM4, "M4")[:RANK]
    FLR = _oem(net, M4, _at(M4, 9), "FLR")[:RANK]

    terms = []
    for i in range(RANK + 1):
        j = RANK - i
        if i == 0:
            terms.append((H5[j - 1][0], H5[j - 1][1] + 4))
        elif j == 0:
            terms.append(FLR[i - 1])
        else:
            (pa, oa) = FLR[i - 1]
            (pb, ob) = H5[j - 1]
            t = net.emit("min", pa, oa, pb, ob + 4, "fin_min")
            terms.append((t, 0))
    while len(terms) > 1:
        nxt = []
        for k in range(0, len(terms) - 1, 2):
            (pa, oa), (pb, ob) = terms[k], terms[k + 1]
            nxt.append((net.emit("max", pa, oa, pb, ob, "fin_max"), 0))
        if len(terms) % 2:
            nxt.append(terms[-1])
        terms = nxt
    ans_plane, ans_off = terms[0]
    assert ans_off == 0
    return net, ans_plane


def _dce(net, ans):
    needed = {ans}
    for op in reversed(net.ops):
        if op.out in needed and op.kind != "leaf":
            needed.add(op.a)
            needed.add(op.b)
    return [op for op in net.ops if op.out in needed]


def _ranges(ops, ans, w_out):
    rng = {ans: (0, w_out)}
    for op in reversed(ops):
        if op.kind == "leaf":
            continue
        lo, hi = rng[op.out]
        for (src, off) in ((op.a, op.ao), (op.b, op.bo)):
            l, h = lo + off, hi + off
            if src in rng:
                l0, h0 = rng[src]
                rng[src] = (min(l0, l), max(h0, h))
            else:
                rng[src] = (l, h)
    return rng


# ------------------------------------------------------------- bass emission


POOL_STAGES = set()      # Pool/GPSIMD cannot run TensorTensor (ISA check)

# DMA groups of dy rows, ordered so T4 (dy0-3) then B4 (dy9-12) convert first
_DMA_GROUPS = [(0, 4), (8, 4), (12, 1), (4, 4)]
_CONV_ORDER = [0, 1, 2, 3, 9, 10, 11, 12, 4, 5, 6, 7, 8]


def _build():
    import concourse.bass as bass
    import concourse.mybir as mybir
    from concourse.ap import AP

    f32 = mybir.dt.float32
    f16 = mybir.dt.float16

    net, ans = _net_build()
    ops = _dce(net, ans)
    rng = _ranges(ops, ans, 512)

    leaf_ops = [op for op in ops if op.kind == "leaf"]
    compute = [op for op in ops if op.kind != "leaf"]
    for op in leaf_ops:
        rng[op.out] = (0, SLOTW)

    producer_stage = {op.out: op.stage for op in ops}
    dve_ops = [op for op in compute if op.stage not in POOL_STAGES]
    pool_ops = [op for op in compute if op.stage in POOL_STAGES]

    # planes produced on DVE but read by a Pool op: pin until Pool is done
    pool_read = set()
    for op in pool_ops:
        for src in (op.a, op.b):
            if producer_stage[src] not in POOL_STAGES:
                pool_read.add(src)

    fin_start = next(
        i for i, op in enumerate(dve_ops) if op.stage in ("fin_min", "fin_max")
    )

    # ---- DVE slot allocation ----
    last_use = {}
    for idx, op in enumerate(dve_ops):
        for src in (op.a, op.b):
            if producer_stage[src] in POOL_STAGES:
                continue
            last_use[src] = max(last_use.get(src, -1), idx)
    for p in pool_read:
        # safe once fin begins: Pool has fully drained by then (s_h5 wait)
        last_use[p] = max(last_use.get(p, -1), fin_start)
    last_use[ans] = len(dve_ops)

    releases = defaultdict(list)
    for p, idx in last_use.items():
        releases[idx].append(p)

    slot_of = {}
    free = []
    next_slot = 0
    for op in leaf_ops:
        slot_of[op.out] = next_slot
        next_slot += 1
    for idx, op in enumerate(dve_ops):
        if free:
            s = free.pop()
        else:
            s = next_slot
            next_slot += 1
        slot_of[op.out] = s
        for p in releases.get(idx, []):
            free.append(slot_of[p])
    n_dve = next_slot

    # ---- Pool slot allocation (separate range; MMM4/H5 outputs pinned) ----
    p_last = {}
    for idx, op in enumerate(pool_ops):
        for src in (op.a, op.b):
            if producer_stage[src] in POOL_STAGES:
                p_last[src] = max(p_last.get(src, -1), idx)
    dve_read = set()
    for op in dve_ops:
        for src in (op.a, op.b):
            if producer_stage[src] in POOL_STAGES:
                dve_read.add(src)
    for p in dve_read:
        p_last[p] = len(pool_ops)          # never reused by Pool

    p_rel = defaultdict(list)
    for p, idx in p_last.items():
        p_rel[idx].append(p)
    p_free = []
    for idx, op in enumerate(pool_ops):
        if p_free:
            s = p_free.pop()
        else:
            s = next_slot
            next_slot += 1
        slot_of[op.out] = s
        for p in p_rel.get(idx, []):
            p_free.append(slot_of[p])

    nslot = next_slot
    arena_w = nslot * SLOTW

    nc = bass.Bass(trn_type="TRN2")
    slab = nc.dram_tensor("slab", [SLAB_H, SLAB_W], f32, kind="ExternalInput")
    out = nc.dram_tensor("out", [128, 512], f32, kind="ExternalOutput")

    REP_W = 13 * SLAB_W

    with (
        nc.sbuf_tensor([128, arena_w], f16) as arena,
        nc.sbuf_tensor([128, REP_W], f32) as rep,
        nc.sbuf_tensor([128, 512], f32) as ansf32,
        nc.semaphore() as dma_sem,
        nc.semaphore() as act_sem,
        nc.semaphore() as dve_sem,
        nc.semaphore() as s_vsort,
        nc.semaphore() as s_h4,
        nc.semaphore() as s_mmm4,
        nc.semaphore() as s_h5,
        nc.Block() as block,
    ):

        def rd(p, off, lo_c, hi_c):
            a_lo, _ = rng[p]
            return AP(
                tensor=arena,
                offset=slot_of[p] * SLOTW + lo_c + off - a_lo,
                ap=[[arena_w, 128], [1, hi_c - lo_c]],
            )

        def wr(p):
            lo, hi = rng[p]
            return AP(
                tensor=arena,
                offset=slot_of[p] * SLOTW,
                ap=[[arena_w, 128], [1, hi - lo]],
            )

        @block.sync
        def _(sync):
            for g0, gc in _DMA_GROUPS:
                src = AP(
                    tensor=slab,
                    offset=g0 * SLAB_W,
                    ap=[[SLAB_W, 128], [SLAB_W, gc], [1, SLAB_W]],
                )
                dst = AP(
                    tensor=rep,
                    offset=g0 * SLAB_W,
                    ap=[[REP_W, 128], [SLAB_W, gc], [1, SLAB_W]],
                )
                sync.dma_start(dst, src).then_inc(dma_sem, 16)
            sync.wait_ge(dve_sem, 1)
            sync.dma_start(out[:, :], ansf32[:, :]).then_inc(dma_sem, 16)

        @block.scalar
        def _(scalar):
            leaf_of = {net.leaf_meta[op.out]: op for op in leaf_ops}
            grp_end = {}
            done = 0
            for gi, (g0, gc) in enumerate(_DMA_GROUPS):
                done += 16
                for dy in range(g0, g0 + gc):
                    grp_end[dy] = done
            for ci, dy in enumerate(_CONV_ORDER):
                scalar.wait_ge(dma_sem, grp_end[dy])
                op = leaf_of[dy]
                ins = nc.scalar.activation(
                    out=wr(op.out),
                    in_=AP(
                        tensor=rep,
                        offset=dy * SLAB_W,
                        ap=[[REP_W, 128], [1, SLAB_W]],
                    ),
                    func=mybir.ActivationFunctionType.Copy,
                    scale=ALPHA,
                )
                ins.then_inc(act_sem, 1)

        alu = {"max": mybir.AluOpType.max, "min": mybir.AluOpType.min}

        def emit(eng, op):
            lo, hi = rng[op.out]
            return eng.tensor_tensor(
                out=wr(op.out),
                in0=rd(op.a, op.ao, lo, hi),
                in1=rd(op.b, op.bo, lo, hi),
                op=alu[op.kind],
            )

        @block.vector
        def _(vector):
            # vsort creation order: T4 (D0-3), B4 (D9-12), M5 (D4-8)
            n_t4 = 10
            n_b4 = 10
            waited = set()
            last = None
            for idx, op in enumerate(dve_ops):
                if idx == 0:
                    vector.wait_ge(act_sem, 4)
                elif idx == n_t4 and "b4" not in waited:
                    vector.wait_ge(act_sem, 8)
                    waited.add("b4")
                elif idx == n_t4 + n_b4 and "m5" not in waited:
                    vector.wait_ge(act_sem, 13)
                    waited.add("m5")
                if pool_ops and op.stage == "M4" and "mmm4" not in waited:
                    vector.wait_ge(s_mmm4, 1)
                    waited.add("mmm4")
                if (
                    pool_ops
                    and op.stage in ("fin_min", "fin_max")
                    and "h5" not in waited
                ):
                    vector.wait_ge(s_h5, 1)
                    waited.add("h5")
                ins = emit(nc.vector, op)
                if op.stage == "vsort" and (
                    idx + 1 == len([o for o in dve_ops if o.stage == "vsort"])
                ):
                    ins.then_inc(s_vsort, 1)
                if op.stage == "H4" and (
                    idx + 1 < len(dve_ops) and dve_ops[idx + 1].stage != "H4"
                ):
                    ins.then_inc(s_h4, 1)
            ins = nc.vector.tensor_copy(ansf32[:, :], rd(ans, 0, 0, 512))
            ins.then_inc(dve_sem, 1)

        if pool_ops:
            @block.gpsimd
            def _(pool):
                pool.wait_ge(s_vsort, 1)
                waited = set()
                for idx, op in enumerate(pool_ops):
                    if op.stage == "H5" and "h4" not in waited:
                        pool.wait_ge(s_h4, 1)
                        waited.add("h4")
                    ins = emit(nc.gpsimd, op)
                    if op.stage == "MMM4" and (
                        idx + 1 < len(pool_ops) and pool_ops[idx + 1].stage != "MMM4"
                    ):
                        ins.then_inc(s_mmm4, 1)
                ins.then_inc(s_h5, 1)

    return nc


def kernel(data: np.ndarray) -> np.ndarray:
    from concourse.bass_utils import run_bass_kernel_spmd

    img = np.asarray(data, dtype=np.float32)[0]          # [512,1024]
    pad = np.pad(img, PW, mode="wrap")                    # [524,1036]

    if "nc" not in _CACHE:
        _CACHE["nc"] = _build()
    nc = _CACHE["nc"]

    in_maps = []
    for c in range(8):
        band, half = c // 2, c % 2
        rb, cb = band * 128, half * 512
        in_maps.append(
            {"slab": np.ascontiguousarray(pad[rb: rb + SLAB_H, cb: cb + SLAB_W])}
        )

    res = run_bass_kernel_spmd(nc, in_maps, core_ids=list(range(8)))

    full = np.empty((V, R), dtype=np.float32)
    for c in range(8):
        band, half = c // 2, c % 2
        full[band * 128: (band + 1) * 128, half * 512: (half + 1) * 512] = (
            res.results[c]["out"]
        )
    return full


# revision 12
# speedup vs baseline: 1.1129x; 1.0761x over previous
"""OS-CFAR 2D rank filter (rank-36-of-144 ring) on 8 Trainium2 NeuronCores.

Strategy: spatially shard [512,1024] into 8 tiles of [128,512] (4 row-bands
x 2 col-halves) with 6-wide circular halos.

Per core, instead of per-pixel max8/match_replace rounds (vector-engine
bound at ~1890ns/pixel-col), compute the order statistic with a shared
sorted-column merge network operating on "planes" (dy-shifted fp16 images),
where every elementwise min/max instruction processes all 128x524 pixels at
once and fp16 gets the DVE 2x perf mode:

  D[dy]    = fp16(ALPHA * slab[p+dy, x])        13 leaf planes
  T4/B4/M5 = vertical sorts of ring column segments (top4/bot4/mid5)
  S8h      = merge(T4,B4)      sorted holed column (8)
  H2/H4    = dyadic merges of holed columns at x..x+3
  H5       = top36 of holed cols x..x+4 = merge(H4, S8h@4)
  MM2/MMM4 = dyadic merges of mid segments
  M4       = top36 of full cols x..x+3 = merge(H4, MMM4)
  FLR      = top36 of the 8 full cols = merge(M4, M4@9)
  ans      = rank36(FLR u H5@4) via max_{i+j=36} min(FLR_i, H5_j)

The rank-36 value of the fp16-rounded multiset equals fp16(rank-36 of the
originals) because rounding is monotone; rel err <= 2^-11 vs the f32
reference, far inside the 2e-2 gate.
"""

import math
from collections import defaultdict
from dataclasses import dataclass

import numpy as np

# ---------------------------------------------------------------- constants
G = (2, 2)
T = (4, 4)
PFA = 1e-05
K = 108
N = 144
PW = 6
V, R = 512, 1024
SLAB_H, SLAB_W = 140, 524      # 128 + 2*PW, 512 + 2*PW
RANK = 36                      # need the 36th largest of the 144 ring cells
SLOTW = 524


def _log_factorial(n):
    n = n + 1
    if n < 9:
        return np.log(float(math.factorial(n)))
    return 0.5 * (np.log(2 * np.pi) - np.log(n)) + n * (
        np.log(n + 1.0 / (12.0 * n - 1.0 / (10.0 * n))) - 1.0
    )


def _fun(k, n, t, pfa):
    return (
        _log_factorial(n)
        - _log_factorial(n - k)
        - np.sum(np.log(np.arange(n, n - k, -1) + t))
        - np.log(pfa)
    )


def _os_cfar_threshold(k, n, pfa):
    lo, hi = 1.0, 1e32
    for _ in range(300):
        mid = 0.5 * (lo + hi)
        if _fun(k, n, mid, pfa) > 0:
            lo = mid
        else:
            hi = mid
    return 0.5 * (lo + hi)


ALPHA = float(np.float32(_os_cfar_threshold(K, N, PFA)))

_CACHE = {}

# ------------------------------------------------------- network generation


@dataclass
class _Op:
    kind: str                 # 'min' | 'max' | 'leaf'
    out: int
    a: int = -1
    ao: int = 0
    b: int = -1
    bo: int = 0
    stage: str = ""


class _Net:
    def __init__(self):
        self.ops = []
        self.n_planes = 0
        self.leaf_meta = {}

    def leaf(self, dy):
        p = self.n_planes
        self.n_planes += 1
        self.leaf_meta[p] = dy
        self.ops.append(_Op("leaf", p, stage="leaf"))
        return p

    def emit(self, kind, a, ao, b, bo, stage):
        p = self.n_planes
        self.n_planes += 1
        self.ops.append(_Op(kind, p, a, ao, b, bo, stage))
        return p

    def ce(self, a, b, stage):
        hi = self.emit("max", a, 0, b, 0, stage)
        lo = self.emit("min", a, 0, b, 0, stage)
        return hi, lo


def _sort_net(net, planes, stage):
    n = len(planes)
    pairs = {
        4: [(0, 1), (2, 3), (0, 2), (1, 3), (1, 2)],
        5: [(0, 1), (3, 4), (2, 4), (2, 3), (0, 3), (0, 2), (1, 4), (1, 3), (1, 2)],
    }[n]
    w = list(planes)
    for i, j in pairs:
        hi, lo = net.ce(w[i], w[j], stage)
        w[i], w[j] = hi, lo
    return w


def _oem(net, A, B, stage):
    if not A:
        return list(B)
    if not B:
        return list(A)
    if len(A) == 1 and len(B) == 1:
        (pa, oa), (pb, ob) = A[0], B[0]
        hi = net.emit("max", pa, oa, pb, ob, stage)
        lo = net.emit("min", pa, oa, pb, ob, stage)
        return [(hi, 0), (lo, 0)]
    E = _oem(net, A[0::2], B[0::2], stage)
    O = _oem(net, A[1::2], B[1::2], stage)
    res = [E[0]]
    n_pairs = min(len(O), len(E) - 1)
    for i in range(n_pairs):
        (po, oo), (pe, oe) = O[i], E[i + 1]
        hi = net.emit("max", po, oo, pe, oe, stage)
        lo = net.emit("min", po, oo, pe, oe, stage)
        res += [(hi, 0), (lo, 0)]
    if len(O) > n_pairs:
        res += O[n_pairs:]
    else:
        res += E[n_pairs + 1:]
    return res


def _at(planes, off):
    return [(p, o + off) for (p, o) in planes]


def _net_build():
    net = _Net()
    D = [net.leaf(dy) for dy in range(13)]

    T4 = _sort_net(net, D[0:4], "vsort")
    B4 = _sort_net(net, D[9:13], "vsort")
    M5 = _sort_net(net, D[4:9], "vsort")
    S8h = _oem(net, [(p, 0) for p in T4], [(p, 0) for p in B4], "vs8")
    M5l = [(p, 0) for p in M5]

    H2 = _oem(net, S8h, _at(S8h, 1), "H2")
    H4 = _oem(net, H2, _at(H2, 2), "H4")
    H5 = _oem(net, H4, _at(S8h, 4), "H5")[:RANK]

    MM2 = _oem(net, M5l, _at(M5l, 1), "MM2")
    MMM4 = _oem(net, MM2, _at(MM2, 2), "MMM4")
    M4 = _oem(net, H4, MMM4, "M4")[:RANK]
    FLR = _oem(net, M4, _at(M4, 9), "FLR")[:RANK]

    terms = []
    for i in range(RANK + 1):
        j = RANK - i
        if i == 0:
            terms.append((H5[j - 1][0], H5[j - 1][1] + 4))
        elif j == 0:
            terms.append(FLR[i - 1])
        else:
            (pa, oa) = FLR[i - 1]
            (pb, ob) = H5[j - 1]
            t = net.emit("min", pa, oa, pb, ob + 4, "fin_min")
            terms.append((t, 0))
    while len(terms) > 1:
        nxt = []
        for k in range(0, len(terms) - 1, 2):
            (pa, oa), (pb, ob) = terms[k], terms[k + 1]
            nxt.append((net.emit("max", pa, oa, pb, ob, "fin_max"), 0))
        if len(terms) % 2:
            nxt.append(terms[-1])
        terms = nxt
    ans_plane, ans_off = terms[0]
    assert ans_off == 0
    return net, ans_plane


def _dce(net, ans):
    needed = {ans}
    for op in reversed(net.ops):
        if op.out in needed and op.kind != "leaf":
            needed.add(op.a)
            needed.add(op.b)
    return [op for op in net.ops if op.out in needed]


def _ranges(ops, ans, w_out):
    rng = {ans: (0, w_out)}
    for op in reversed(ops):
        if op.kind == "leaf":
            continue
        lo, hi = rng[op.out]
        for (src, off) in ((op.a, op.ao), (op.b, op.bo)):
            l, h = lo + off, hi + off
            if src in rng:
                l0, h0 = rng[src]
                rng[src] = (min(l0, l), max(h0, h))
            else:
                rng[src] = (l, h)
    return rng


# ------------------------------------------------------------- bass emission


POOL_STAGES = set()      # Pool/GPSIMD cannot run TensorTensor (ISA check)

# DMA groups of dy rows, ordered so T4 (dy0-3) then B4 (dy9-12) convert first
_DMA_GROUPS = [(0, 4), (8, 4), (12, 1), (4, 4)]
_CONV_ORDER = [0, 1, 2, 3, 9, 10, 11, 12, 4, 5, 6, 7, 8]


def _build():
    import concourse.bass as bass
    import concourse.mybir as mybir
    from concourse.ap import AP

    f32 = mybir.dt.float32
    f16 = mybir.dt.float16

    net, ans = _net_build()
    ops = _dce(net, ans)
    rng = _ranges(ops, ans, 512)

    leaf_ops = [op for op in ops if op.kind == "leaf"]
    compute = [op for op in ops if op.kind != "leaf"]
    for op in leaf_ops:
        rng[op.out] = (0, SLOTW)

    producer_stage = {op.out: op.stage for op in ops}
    dve_ops = list(compute)
    pool_ops = []

    vsort_ops = [op for op in dve_ops if op.stage == "vsort"]
    rest_ops = [op for op in dve_ops if op.stage != "vsort"]

    # ---- ASAP levels (vsort + leaves at level 0) ----
    level = {op.out: 0 for op in leaf_ops}
    for op in vsort_ops:
        level[op.out] = 0
    stage_rank = {
        s: i
        for i, s in enumerate(
            ["vs8", "H2", "H4", "H5", "MM2", "MMM4", "M4", "FLR",
             "fin_min", "fin_max"]
        )
    }
    for op in rest_ops:
        level[op.out] = 1 + max(level[op.a], level[op.b])
    order = sorted(
        range(len(rest_ops)),
        key=lambda i: (
            stage_rank[rest_ops[i].stage],
            level[rest_ops[i].out],
            rest_ops[i].kind,
            i,
        ),
    )
    rest_ops = [rest_ops[i] for i in order]

    # groups: consecutive runs with same (stage, level, kind)
    groups = []
    for op in rest_ops:
        key = (op.stage, level[op.out], op.kind)
        if groups and groups[-1][0] == key:
            groups[-1][1].append(op)
        else:
            groups.append((key, [op]))

    # ---- positions for lifetime analysis ----
    # vsort op j -> position j; group g -> position len(vsort_ops)+g
    pos_of_group = {}
    n_positions = len(vsort_ops)
    for gi, (_, gops) in enumerate(groups):
        for op in gops:
            pos_of_group[op.out] = n_positions + gi
    n_groups = len(groups)

    last_use = {}
    for j, op in enumerate(vsort_ops):
        for src in (op.a, op.b):
            last_use[src] = max(last_use.get(src, -1), j)
    for gi, (_, gops) in enumerate(groups):
        p = len(vsort_ops) + gi
        for op in gops:
            for src in (op.a, op.b):
                last_use[src] = max(last_use.get(src, -1), p)
    last_use[ans] = len(vsort_ops) + n_groups + 1

    releases = defaultdict(list)
    for p, idx in last_use.items():
        releases[idx].append(p)

    # ---- range-based slot allocator ----
    free_ranges: list[list[int]] = []     # sorted [start, end)
    next_slot = 0
    slot_of = {}

    def _coalesce():
        free_ranges.sort()
        merged = []
        for r in free_ranges:
            if merged and merged[-1][1] == r[0]:
                merged[-1][1] = r[1]
            else:
                merged.append(r)
        free_ranges[:] = merged

    def alloc_chunks(k):
        """k slots as few contiguous chunks; smallest-fit, else largest-first."""
        nonlocal next_slot
        chunks = []
        best = None
        for r in free_ranges:
            sz = r[1] - r[0]
            if sz >= k and (best is None or sz < best[1] - best[0]):
                best = r
        if best is not None:
            chunks.append((best[0], k))
            best[0] += k
            if best[0] == best[1]:
                free_ranges.remove(best)
            return chunks
        while k > 0 and free_ranges:
            r = max(free_ranges, key=lambda r: r[1] - r[0])
            take = min(k, r[1] - r[0])
            chunks.append((r[0], take))
            r[0] += take
            if r[0] == r[1]:
                free_ranges.remove(r)
            k -= take
        if k > 0:
            chunks.append((next_slot, k))
            next_slot += k
        return chunks

    def free_slot(s):
        free_ranges.append([s, s + 1])
        _coalesce()

    for op in leaf_ops:
        (c,) = alloc_chunks(1)
        slot_of[op.out] = c[0]
    for j, op in enumerate(vsort_ops):
        (c,) = alloc_chunks(1)[:1]
        slot_of[op.out] = c[0]
        for p in releases.get(j, []):
            free_slot(slot_of[p])
    for gi, (_, gops) in enumerate(groups):
        chunks = alloc_chunks(len(gops))
        flat = [c[0] + i for c in chunks for i in range(c[1])]
        for k, op in enumerate(gops):
            slot_of[op.out] = flat[k]
        for p in releases.get(len(vsort_ops) + gi, []):
            free_slot(slot_of[p])

    nslot = next_slot + 1                 # +1 pad slot for width-padded reads
    arena_w = nslot * SLOTW

    nc = bass.Bass(trn_type="TRN2")
    slab = nc.dram_tensor("slab", [SLAB_H, SLAB_W], f32, kind="ExternalInput")
    out = nc.dram_tensor("out", [128, 512], f32, kind="ExternalOutput")

    REP_W = 13 * SLAB_W

    with (
        nc.sbuf_tensor([128, arena_w], f16) as arena,
        nc.sbuf_tensor([128, REP_W], f32) as rep,
        nc.sbuf_tensor([128, 512], f32) as ansf32,
        nc.semaphore() as dma_sem,
        nc.semaphore() as act_sem,
        nc.semaphore() as dve_sem,
        nc.semaphore() as s_vsort,
        nc.semaphore() as s_h4,
        nc.semaphore() as s_mmm4,
        nc.semaphore() as s_h5,
        nc.Block() as block,
    ):

        def rd(p, off, lo_c, hi_c):
            a_lo, _ = rng[p]
            return AP(
                tensor=arena,
                offset=slot_of[p] * SLOTW + lo_c + off - a_lo,
                ap=[[arena_w, 128], [1, hi_c - lo_c]],
            )

        def wr(p):
            lo, hi = rng[p]
            return AP(
                tensor=arena,
                offset=slot_of[p] * SLOTW,
                ap=[[arena_w, 128], [1, hi - lo]],
            )

        @block.sync
        def _(sync):
            for g0, gc in _DMA_GROUPS:
                src = AP(
                    tensor=slab,
                    offset=g0 * SLAB_W,
                    ap=[[SLAB_W, 128], [SLAB_W, gc], [1, SLAB_W]],
                )
                dst = AP(
                    tensor=rep,
                    offset=g0 * SLAB_W,
                    ap=[[REP_W, 128], [SLAB_W, gc], [1, SLAB_W]],
                )
                sync.dma_start(dst, src).then_inc(dma_sem, 16)
            sync.wait_ge(dve_sem, 1)
            sync.dma_start(out[:, :], ansf32[:, :]).then_inc(dma_sem, 16)

        @block.scalar
        def _(scalar):
            leaf_of = {net.leaf_meta[op.out]: op for op in leaf_ops}
            grp_end = {}
            done = 0
            for gi, (g0, gc) in enumerate(_DMA_GROUPS):
                done += 16
                for dy in range(g0, g0 + gc):
                    grp_end[dy] = done
            for ci, dy in enumerate(_CONV_ORDER):
                scalar.wait_ge(dma_sem, grp_end[dy])
                op = leaf_of[dy]
                ins = nc.scalar.activation(
                    out=wr(op.out),
                    in_=AP(
                        tensor=rep,
                        offset=dy * SLAB_W,
                        ap=[[REP_W, 128], [1, SLAB_W]],
                    ),
                    func=mybir.ActivationFunctionType.Copy,
                    scale=ALPHA,
                )
                ins.then_inc(act_sem, 1)

        alu = {"max": mybir.AluOpType.max, "min": mybir.AluOpType.min}

        def op_offsets(op):
            """(out_off, in0_off, in1_off, width) in elements."""
            lo, hi = rng[op.out]
            a_lo = rng[op.a][0]
            b_lo = rng[op.b][0]
            return (
                slot_of[op.out] * SLOTW,
                slot_of[op.a] * SLOTW + lo + op.ao - a_lo,
                slot_of[op.b] * SLOTW + lo + op.bo - b_lo,
                hi - lo,
            )

        @block.vector
        def _(vector):
            # vsort, sequential: T4 (D0-3), B4 (D9-12), M5 (D4-8)
            for idx, op in enumerate(vsort_ops):
                if idx == 0:
                    vector.wait_ge(act_sem, 4)
                elif idx == 10:
                    vector.wait_ge(act_sem, 8)
                elif idx == 20:
                    vector.wait_ge(act_sem, 13)
                lo, hi = rng[op.out]
                nc.vector.tensor_tensor(
                    out=wr(op.out),
                    in0=rd(op.a, op.ao, lo, hi),
                    in1=rd(op.b, op.bo, lo, hi),
                    op=alu[op.kind],
                )

            # batched groups: maximal affine runs -> one instruction each
            for (stage, lvl, kind), gops in groups:
                i = 0
                while i < len(gops):
                    offs = [op_offsets(op) for op in gops[i:]]
                    k = 1
                    if len(offs) > 1:
                        d = tuple(
                            offs[1][c] - offs[0][c] for c in range(3)
                        )
                        if all(abs(x) <= 32000 for x in d):
                            while k < len(offs) and all(
                                offs[k][c] - offs[k - 1][c] == d[c]
                                for c in range(3)
                            ):
                                k += 1
                    w = max(o[3] for o in offs[:k])
                    if k == 1:
                        o0, i0, i1, _ = offs[0]
                        nc.vector.tensor_tensor(
                            out=AP(tensor=arena, offset=o0,
                                   ap=[[arena_w, 128], [1, w]]),
                            in0=AP(tensor=arena, offset=i0,
                                   ap=[[arena_w, 128], [1, w]]),
                            in1=AP(tensor=arena, offset=i1,
                                   ap=[[arena_w, 128], [1, w]]),
                            op=alu[kind],
                        )
                    else:
                        o0, i0, i1, _ = offs[0]
                        do, d0, d1 = d
                        nc.vector.tensor_tensor(
                            out=AP(tensor=arena, offset=o0,
                                   ap=[[arena_w, 128], [do, k], [1, w]]),
                            in0=AP(tensor=arena, offset=i0,
                                   ap=[[arena_w, 128], [d0, k], [1, w]]),
                            in1=AP(tensor=arena, offset=i1,
                                   ap=[[arena_w, 128], [d1, k], [1, w]]),
                            op=alu[kind],
                        )
                    i += k

            ins = nc.vector.tensor_copy(ansf32[:, :], rd(ans, 0, 0, 512))
            ins.then_inc(dve_sem, 1)

    return nc


def kernel(data: np.ndarray) -> np.ndarray:
    from concourse.bass_utils import run_bass_kernel_spmd

    img = np.asarray(data, dtype=np.float32)[0]          # [512,1024]
    pad = np.pad(img, PW, mode="wrap")                    # [524,1036]

    if "nc" not in _CACHE:
        _CACHE["nc"] = _build()
    nc = _CACHE["nc"]

    in_maps = []
    for c in range(8):
        band, half = c // 2, c % 2
        rb, cb = band * 128, half * 512
        in_maps.append(
            {"slab": np.ascontiguousarray(pad[rb: rb + SLAB_H, cb: cb + SLAB_W])}
        )

    res = run_bass_kernel_spmd(nc, in_maps, core_ids=list(range(8)))

    full = np.empty((V, R), dtype=np.float32)
    for c in range(8):
        band, half = c // 2, c % 2
        full[band * 128: (band + 1) * 128, half * 512: (half + 1) * 512] = (
            res.results[c]["out"]
        )
    return full
